# revision 24
# baseline (speedup 1.0000x reference)
"""Trainium2 Bass kernel for nn_BidirectionalRead (N=2, C=256, H=W=64).

Reference (per sample; f1,f2 as [C=256, HW=4096]):
  SA1:  c_i = softmax(cw_i @ f_i + cb_i) over HW;  f_i1 = c_i * f_i
  CA:   G_i = pw_i @ f_i1 + pb_i  [16, HW]
        B[p,q] = tanh(sum_k G1[k,p] G2[k,q]);  A = B^T
        f1_hat = l2norm_c(f1_1 @ B); f2_hat = l2norm_c(f2_1 @ A)
  fp_i  = relu(f_i_hat + f_i)
  SA2:  out_i = softmax(cw_i @ fp_i + cb_i) * fp_i

Sharding: 8 cores = 2 samples x 4 q-blocks of 1024 columns. Inputs are
np.roll'ed per core so identical SPMD code always computes output columns
0..1023 of its rotated view. SA2 softmax denominators are exchanged with
one tiny AllGather per direction over groups [[0-3],[4-7]].

Device layout choices:
 - x streamed in [c, q] spans; "transpose" via regular fp32r matmul
   against [I | cw^T | 0] (N=256) so the SA1 matvec rides along free.
 - Main matmuls emit f_hat TRANSPOSED [q, c]: l2norm / residual / SA2
   matvec / softmax scaling are all free-dim or per-partition ops.
 - All large matmuls in fp32r (full PE rate at N>=256).
"""

import sys

sys.path.insert(0, "/opt/trn_rl_repo")

import numpy as np

import concourse.bass as bass
import concourse.mybir as mybir
import concourse.tile as tile
from concourse import bass_isa, library_config

F32 = mybir.dt.float32
F32R = mybir.dt.float32r

N_CORES = 8
N, C, H, W = 2, 256, 64, 64
HW = H * W              # 4096
P = 128
CH = C // P             # 2 c-chunks
NT = HW // P            # 32 p/q tiles
Q = HW // 4             # 1024 columns per core
QT = Q // P             # 8 q tiles per block
CL = 16
SPAN = 512
EPS = 1e-12

_CACHED = {}


def _build_nc():
    from concourse import bacc

    # Bacc (not raw Bass): its compile() pass splits multi-sem waits that
    # walrus rejects ("Too many sync wait commands").
    nc = bacc.Bacc(trn_type="TRN2", num_devices=N_CORES)
    AF = mybir.ActivationFunctionType
    OP = mybir.AluOpType

    x1_d = nc.declare_dram_parameter("x1", [C, HW], F32, isOutput=False)
    x2_d = nc.declare_dram_parameter("x2", [C, HW], F32, isOutput=False)
    x_d = [x1_d, x2_d]
    aug1_d = nc.declare_dram_parameter("aug1", [P, CH, 2 * P], F32, isOutput=False)
    aug2_d = nc.declare_dram_parameter("aug2", [P, CH, 2 * P], F32, isOutput=False)
    aug_d = [aug1_d, aug2_d]
    pw1_d = nc.declare_dram_parameter("pw1t", [P, CH, CL], F32, isOutput=False)
    pw2_d = nc.declare_dram_parameter("pw2t", [P, CH, CL], F32, isOutput=False)
    pw_d = [pw1_d, pw2_d]
    pb1_d = nc.declare_dram_parameter("pb1c", [CL, 1], F32, isOutput=False)
    pb2_d = nc.declare_dram_parameter("pb2c", [CL, 1], F32, isOutput=False)
    pb_d = [pb1_d, pb2_d]
    cb1_d = nc.declare_dram_parameter("cb1c", [1, 1], F32, isOutput=False)
    cb2_d = nc.declare_dram_parameter("cb2c", [1, 1], F32, isOutput=False)
    cb_d = [cb1_d, cb2_d]
    cwb1_d = nc.declare_dram_parameter("cwb1", [P, C], F32, isOutput=False)
    cwb2_d = nc.declare_dram_parameter("cwb2", [P, C], F32, isOutput=False)
    cwb_d = [cwb1_d, cwb2_d]
    y1_d = nc.declare_dram_parameter("y1", [C, Q], F32, isOutput=True)
    y2_d = nc.declare_dram_parameter("y2", [C, Q], F32, isOutput=True)
    y_d = [y1_d, y2_d]

    groups = [[0, 1, 2, 3], [4, 5, 6, 7]]

    with tile.TileContext(nc) as tc:
        with (
            tc.tile_pool(name="const", bufs=1) as const,
            tc.tile_pool(name="persist", bufs=1) as persist,
            tc.tile_pool(name="small", bufs=1) as small,
            tc.tile_pool(name="dram", bufs=1, space="DRAM") as dram,
        ):
            # ---- constants / weights ----
            aug = [const.tile([P, CH, 2 * P], F32, name=f"aug{i}", tag=f"aug{i}")
                   for i in range(2)]
            pwt = [const.tile([P, CH, CL], F32, name=f"pwt{i}", tag=f"pwt{i}")
                   for i in range(2)]
            pbc = [const.tile([CL, 1], F32, name=f"pbc{i}", tag=f"pbc{i}")
                   for i in range(2)]
            cbb = [const.tile([P, 1], F32, name=f"cbb{i}", tag=f"cbb{i}")
                   for i in range(2)]
            cwb = [const.tile([P, C], F32, name=f"cwb{i}", tag=f"cwb{i}")
                   for i in range(2)]
            ones_c = const.tile([P, 1], F32, name="ones_c", tag="ones_c")
            ones_r = const.tile([1, P], F32, name="ones_r", tag="ones_r")
            nc.gpsimd.memset(ones_c[:], 1.0)
            nc.gpsimd.memset(ones_r[:], 1.0)
            for i in range(2):
                nc.sync.dma_start(aug[i][:], aug_d[i][:])
                nc.sync.dma_start(pwt[i][:], pw_d[i][:])
                nc.sync.dma_start(pbc[i][:], pb_d[i][:])
                nc.sync.dma_start(cbb[i][:], cb_d[i][:].to_broadcast((P, 1)))
                nc.sync.dma_start(cwb[i][:], cwb_d[i][:])
            ident = aug[0][:, 0, 0:P]  # [128, 128] exact identity

            # ---- persistent buffers ----
            xT = [persist.tile([P, NT, C], F32, name=f"x{i}T", tag=f"x{i}T")
                  for i in range(2)]
            xR = [persist.tile([P, QT, C], F32, name=f"x{i}R", tag=f"x{i}R")
                  for i in range(2)]
            Gp = [persist.tile([CL, HW], F32, name=f"G{i}p", tag=f"G{i}p")
                  for i in range(2)]
            fpT = [persist.tile([P, QT, C], F32, name=f"fp{i}T", tag=f"fp{i}T")
                   for i in range(2)]
            e2t = [persist.tile([P, QT], F32, name=f"e2t{i}", tag=f"e2t{i}")
                   for i in range(2)]

            # ================= Phase A =================
            with (
                tc.tile_pool(name="pa_sb", bufs=1) as pa,
                tc.tile_pool(name="pa_ps", bufs=1, space="PSUM") as pa_ps,
            ):
                # fp32r-rounded copy of pw^T (verifier: matmul operands
                # must come from compute-engine producers w/ f32r out)
                pwr = [pa.tile([P, CH, CL], F32R, name=f"pwr{i}",
                               tag=f"pwr{i}", bufs=1) for i in range(2)]
                for i in range(2):
                    nc.vector.tensor_copy(pwr[i][:], pwt[i][:])
                for i in range(2):
                    craw = pa.tile([P, NT], F32, name=f"craw{i}", tag="craw",
                                   bufs=2)
                    for s in range(HW // SPAN):
                        xs = pa.tile([P, CH, SPAN], F32, name="xspan",
                                     tag="xspan", bufs=3)
                        nc.sync.dma_start(
                            xs[:],
                            x_d[i][:]
                            .rearrange("(a p) q -> p a q", p=P)[
                                :, :, s * SPAN:(s + 1) * SPAN
                            ],
                        )
                        # rounded copy for the fp32r Graw matmul (gpsimd
                        # is idle in phase A — free)
                        xsr = pa.tile([P, CH, SPAN], F32R, name="xspanr",
                                      tag="xspanr", bufs=3)
                        nc.gpsimd.tensor_copy(xsr[:], xs[:])
                        # Graw chunk for this span (accumulate c-chunks)
                        gps = pa_ps.tile([CL, SPAN], F32, name="gps",
                                         tag="gps", bufs=2)
                        for a in range(CH):
                            nc.tensor.matmul(
                                gps[:],
                                pwr[i][:, a, :],
                                xsr[:, a, :],
                                start=(a == 0),
                                stop=(a == CH - 1),
                            )
                        nc.vector.tensor_copy(
                            Gp[i][:, s * SPAN:(s + 1) * SPAN].bitcast(F32R),
                            gps[:],
                        )
                        # transposes (exact, transpose-mode fp32)
                        for tl in range(SPAN // P):
                            t = s * (SPAN // P) + tl
                            tps = [
                                pa_ps.tile([P, P], F32, name="tps",
                                           tag="tps", bufs=4)
                                for _ in range(CH)
                            ]
                            for a in range(CH):
                                nc.tensor.transpose(
                                    tps[a][:],
                                    xs[:, a, tl * P:(tl + 1) * P],
                                    ident,
                                )
                                nc.vector.tensor_copy(
                                    xT[i][:, t, a * P:(a + 1) * P]
                                    .bitcast(F32R),
                                    tps[a][:, 0:P],
                                )
                                if t < QT:
                                    # exact fp32 residual copy from PSUM
                                    nc.vector.tensor_copy(
                                        xR[i][:, t, a * P:(a + 1) * P],
                                        tps[a][:, 0:P],
                                    )
                            # SA1 matvec: craw[:, t] = sum_c x^T[q,c]*cw[c]
                            cscr = pa.tile([P, C], F32, name="cscr",
                                           tag="cscr", bufs=2)
                            nc.vector.scalar_tensor_tensor(
                                out=cscr[:], in0=xT[i][:, t, :],
                                scalar=1.0, in1=cwb[i][:],
                                op0=OP.mult, op1=OP.mult,
                                accum_out=craw[:, t:t + 1],
                            )
                    # SA1 softmax (no max subtraction; craw ~ N(0,1))
                    e1 = pa.tile([P, NT], F32, name="e1", tag="e1", bufs=2)
                    es = pa.tile([P, 1], F32, name="es", tag="es", bufs=2)
                    nc.scalar.activation(
                        e1[:], craw[:], AF.Exp, bias=cbb[i][:], scale=1.0,
                        accum_out=es[:],
                    )
                    # partition-sum via ones-matmul, then K=1 re-broadcast
                    s_ps = pa_ps.tile([1, 1], F32, name="s_ps", tag="c1T",
                                      bufs=2)
                    nc.tensor.matmul(s_ps[:], ones_c[:], es[:])
                    rsc = pa.tile([1, 1], F32, name="rsc", tag="rsc", bufs=2)
                    nc.vector.reciprocal(rsc[:], s_ps[:])
                    rb_ps = pa_ps.tile([P, 1], F32, name="rb_ps", tag="c1T",
                                       bufs=2)
                    nc.tensor.matmul(rb_ps[:], ones_r[:], rsc[:])
                    rs = pa.tile([P, 1], F32, name="rs", tag="rs", bufs=2)
                    nc.vector.tensor_copy(rs[:], rb_ps[:])
                    c1t = pa.tile([P, NT], F32, name="c1t", tag="c1t", bufs=2)
                    nc.vector.tensor_scalar_mul(c1t[:], e1[:], rs[:])

                    # in-place scale; f32r out dtype doubles as the
                    # verifier-required rounding for the main matmuls
                    for t in range(NT):
                        nc.vector.tensor_scalar_mul(
                            xT[i][:, t, :].bitcast(F32R),
                            xT[i][:, t, :], c1t[:, t:t + 1]
                        )

                    # c1 row (DRAM bounce) -> 16-partition broadcast
                    c1T_ps = pa_ps.tile([32, P], F32, name="c1Tps", tag="c1T",
                                        bufs=2)
                    nc.tensor.transpose(c1T_ps[:], c1t[:], ident)
                    c1T = pa.tile([32, P], F32, name="c1T", tag="c1Tsb",
                                  bufs=2)
                    nc.vector.tensor_copy(c1T[:], c1T_ps[:])
                    c1row_d = dram.tile([1, HW], F32, name=f"c1row{i}",
                                        tag=f"c1row{i}")
                    nc.sync.dma_start(
                        c1row_d[:].rearrange("o (t r) -> (o t) r", t=32),
                        c1T[:],
                    )
                    c1bc = pa.tile([CL, HW], F32, name="c1bc", tag="c1bc",
                                   bufs=2)
                    nc.sync.dma_start(
                        c1bc[:], c1row_d[:].to_broadcast((CL, HW))
                    )
                    # G' = Graw * c1 + pb  (in-place on Gp)
                    nc.vector.tensor_tensor(
                        out=Gp[i][:].bitcast(F32R), in0=Gp[i][:],
                        in1=c1bc[:], op=OP.mult,
                    )
                    nc.vector.tensor_scalar_add(
                        Gp[i][:].bitcast(F32R), Gp[i][:], pbc[i][:]
                    )

            # ================= Phase B + C =================
            with (
                tc.tile_pool(name="pb_sb", bufs=1) as pb,
                tc.tile_pool(name="pb_ps", bufs=1, space="PSUM") as pb_ps,
                tc.tile_pool(name="pc_ps", bufs=1, space="PSUM") as pc_ps,
            ):
                ysb = [pb.tile([P, CH, Q], F32, name=f"y{i}sb", tag=f"y{i}sb")
                       for i in range(2)]
                got = [pb.tile([4, 8], F32, name=f"got{d}", tag=f"got{d}")
                       for d in range(2)]
                for d in range(2):
                    gL, gR = (Gp[0], Gp[1]) if d == 0 else (Gp[1], Gp[0])
                    acc = [
                        pb_ps.tile([P, C], F32, name=f"acc{j}", tag=f"acc{j}",
                                   bufs=1)
                        for j in range(SPAN // P)
                    ]
                    craw2 = pb.tile([P, QT], F32, name=f"craw2_{d}",
                                    tag="craw2", bufs=2)
                    for s in range(Q // SPAN):
                        for p in range(NT):
                            aps = pb_ps.tile([P, SPAN], F32, name="aps",
                                             tag="aps", bufs=2)
                            nc.tensor.matmul(
                                aps[:],
                                gL[:, p * P:(p + 1) * P].bitcast(F32R),
                                gR[:, s * SPAN:(s + 1) * SPAN].bitcast(F32R),
                            )
                            asb = pb.tile([P, SPAN], F32R, name="asb",
                                          tag="asb", bufs=3)
                            nc.scalar.activation(asb[:], aps[:], AF.Tanh)
                            for j in range(SPAN // P):
                                nc.tensor.matmul(
                                    acc[j][:],
                                    asb[:, j * P:(j + 1) * P],
                                    xT[d][:, p, :].bitcast(F32R),
                                    start=(p == 0),
                                    stop=(p == NT - 1),
                                )
                        for j in range(SPAN // P):
                            qt = s * (SPAN // P) + j
                            sq = pb.tile([P, C], F32, name="sq", tag="sq",
                                         bufs=2)
                            ss = pb.tile([P, 1], F32, name="ss", tag="ss",
                                         bufs=2)
                            nc.scalar.activation(
                                sq[:], acc[j][:], AF.Square,
                                accum_out=ss[:],
                            )
                            nrm = pb.tile([P, 1], F32, name="nrm", tag="nrm",
                                          bufs=2)
                            nc.scalar.sqrt(nrm[:], ss[:])
                            nc.vector.tensor_scalar_max(nrm[:], nrm[:], EPS)
                            rn = pb.tile([P, 1], F32, name="rn", tag="rn",
                                         bufs=2)
                            nc.vector.reciprocal(rn[:], nrm[:])
                            tres = pb.tile([P, C], F32, name="tres",
                                           tag="tres", bufs=2)
                            nc.vector.scalar_tensor_tensor(
                                out=tres[:], in0=acc[j][:], scalar=rn[:],
                                in1=xR[d][:, qt, :], op0=OP.mult, op1=OP.add,
                            )
                            nc.scalar.activation(
                                fpT[d][:, qt, :], tres[:], AF.Relu
                            )
                            sq2 = pb.tile([P, C], F32, name="sq2", tag="sq",
                                          bufs=2)
                            nc.vector.scalar_tensor_tensor(
                                out=sq2[:], in0=fpT[d][:, qt, :],
                                scalar=1.0, in1=cwb[d][:],
                                op0=OP.mult, op1=OP.mult,
                                accum_out=craw2[:, qt:qt + 1],
                            )
                    # SA2 local stats + AllGather
                    es2 = pb.tile([P, 1], F32, name="es2", tag="es2", bufs=2)
                    nc.scalar.activation(
                        e2t[d][:], craw2[:], AF.Exp, bias=cbb[d][:],
                        scale=1.0, accum_out=es2[:],
                    )
                    s2_ps = pc_ps.tile([1, 1], F32, name="s2_ps", tag="ytp",
                                       bufs=2)
                    nc.tensor.matmul(s2_ps[:], ones_c[:], es2[:])
                    pad = pb.tile([1, 8], F32, name="pad", tag="pad", bufs=2)
                    nc.gpsimd.memset(pad[:], 0.0)
                    nc.vector.tensor_copy(pad[:, 0:1], s2_ps[:])
                    cc_in = dram.tile([1, 8], F32, name=f"ccin{d}",
                                      tag=f"ccin{d}")
                    cc_out = dram.tile([4, 8], F32, name=f"ccout{d}",
                                       tag=f"ccout{d}")
                    nc.sync.dma_start(cc_in[:], pad[:])
                    nc.gpsimd.collective_compute(
                        "AllGather", mybir.AluOpType.bypass,
                        replica_groups=groups, ins=[cc_in[:]],
                        outs=[cc_out[:]],
                    )
                    nc.sync.dma_start(got[d][:], cc_out[:])

                # ---- finalize ----
                for d in range(2):
                    stot_ps = pc_ps.tile([1, 8], F32, name=f"stot{d}",
                                         tag="ytp", bufs=2)
                    nc.tensor.matmul(stot_ps[:], ones_c[0:4, :], got[d][:])
                    rq = pb.tile([1, 1], F32, name=f"rq{d}", tag="rq", bufs=2)
                    nc.vector.reciprocal(rq[:], stot_ps[:, 0:1])
                    rb_d = dram.tile([1, 1], F32, name=f"rb{d}", tag=f"rb{d}")
                    nc.sync.dma_start(rb_d[:], rq[:])
                    rb = pb.tile([P, 1], F32, name=f"rb{d}", tag="rb", bufs=2)
                    nc.sync.dma_start(rb[:], rb_d[:].to_broadcast((P, 1)))
                    scl = pb.tile([P, QT], F32, name=f"scl{d}", tag="scl",
                                  bufs=2)
                    nc.vector.tensor_scalar_mul(scl[:], e2t[d][:], rb[:])
                    for qt in range(QT):
                        fin = pb.tile([P, C], F32, name="fin", tag="fin",
                                      bufs=3)
                        nc.vector.tensor_scalar_mul(
                            fin[:], fpT[d][:, qt, :], scl[:, qt:qt + 1]
                        )
                        for a in range(CH):
                            ytp = pc_ps.tile([P, P], F32, name="ytp",
                                             tag="ytp", bufs=2)
                            nc.tensor.transpose(
                                ytp[:], fin[:, a * P:(a + 1) * P], ident
                            )
                            nc.vector.tensor_copy(
                                ysb[d][:, a, qt * P:(qt + 1) * P], ytp[:]
                            )
                    for a in range(CH):
                        nc.sync.dma_start(
                            y_d[d][a * P:(a + 1) * P, :], ysb[d][:, a, :]
                        )
    if not nc.is_finalized():
        nc.finalize()
    return nc


def _get_nc():
    if "nc" not in _CACHED:
        _CACHED["nc"] = _build_nc()
    return _CACHED["nc"]


def _prep_inputs(f1, f2, pw1, pb1, pw2, pb2, cw1, cb1, cw2, cb2):
    f1 = np.asarray(f1, np.float32).reshape(N, C, HW)
    f2 = np.asarray(f2, np.float32).reshape(N, C, HW)

    def augment(cw):
        a = np.zeros((P, CH, 2 * P), np.float32)
        cw = np.asarray(cw, np.float32).reshape(C)
        for c in range(CH):
            a[:, c, 0:P] = np.eye(P, dtype=np.float32)
            a[:, c, P] = cw[c * P:(c + 1) * P]
        return a

    def pwt(pw):
        w = np.asarray(pw, np.float32).reshape(CL, CH, P)  # [k, a, p]
        return np.ascontiguousarray(w.transpose(2, 1, 0))  # [p, a, k]

    common = {
        "aug1": augment(cw1), "aug2": augment(cw2),
        "pw1t": pwt(pw1), "pw2t": pwt(pw2),
        "pb1c": np.asarray(pb1, np.float32).reshape(CL, 1),
        "pb2c": np.asarray(pb2, np.float32).reshape(CL, 1),
        "cb1c": np.asarray(cb1, np.float32).reshape(1, 1),
        "cb2c": np.asarray(cb2, np.float32).reshape(1, 1),
        "cwb1": np.broadcast_to(
            np.asarray(cw1, np.float32).reshape(1, C), (P, C)
        ).copy(),
        "cwb2": np.broadcast_to(
            np.asarray(cw2, np.float32).reshape(1, C), (P, C)
        ).copy(),
    }
    in_maps = []
    for core in range(N_CORES):
        n, b = divmod(core, 4)
        m = dict(common)
        m["x1"] = np.ascontiguousarray(np.roll(f1[n], -b * Q, axis=1))
        m["x2"] = np.ascontiguousarray(np.roll(f2[n], -b * Q, axis=1))
        in_maps.append(m)
    return in_maps


def _assemble(results):
    o1 = np.empty((N, C, HW), np.float32)
    o2 = np.empty((N, C, HW), np.float32)
    for core in range(N_CORES):
        n, b = divmod(core, 4)
        o1[n][:, b * Q:(b + 1) * Q] = results[core]["y1"]
        o2[n][:, b * Q:(b + 1) * Q] = results[core]["y2"]
    return o1.reshape(N, C, H, W), o2.reshape(N, C, H, W)


def run(trace=False, **inputs):
    from concourse.bass_utils import run_bass_kernel_spmd

    nc = _get_nc()
    in_maps = _prep_inputs(**inputs)
    res = run_bass_kernel_spmd(
        nc, in_maps, core_ids=list(range(N_CORES)), trace=trace
    )
    return _assemble(res.results), res


def kernel(**inputs):
    out, _ = run(trace=False, **inputs)
    return out


# revision 27
# speedup vs baseline: 1.2467x; 1.2467x over previous
"""Trainium2 Bass kernel for nn_BidirectionalRead (N=2, C=256, H=W=64).

Reference (per sample; f1,f2 as [C=256, HW=4096]):
  SA1:  c_i = softmax(cw_i @ f_i + cb_i) over HW;  f_i1 = c_i * f_i
  CA:   G_i = pw_i @ f_i1 + pb_i  [16, HW]
        B[p,q] = tanh(sum_k G1[k,p] G2[k,q]);  A = B^T
        f1_hat = l2norm_c(f1_1 @ B); f2_hat = l2norm_c(f2_1 @ A)
  fp_i  = relu(f_i_hat + f_i)
  SA2:  out_i = softmax(cw_i @ fp_i + cb_i) * fp_i

Sharding: 8 cores = 2 samples x 4 q-blocks of 1024 columns. Inputs are
np.roll'ed per core so identical SPMD code always computes output columns
0..1023 of its rotated view. SA2 softmax denominators are exchanged with
one tiny AllGather per direction over groups [[0-3],[4-7]].

Layout / precision choices (v2):
 - x transposed tile-by-tile on the PE (transpose-mode fp32, exact);
   the raw residual block is copied from PSUM in fp32, everything that
   feeds the big matmuls is converted to bf16 (full PE rate + FWL).
 - Main matmuls emit f_hat TRANSPOSED [q, c]: l2norm / residual / SA2
   matvec / softmax scaling are all free-dim or per-partition ops.
 - tanh runs on ACT over merged [128, 2x512] pairs to amortize the
   352-cycle per-instruction overhead.
"""

import sys

sys.path.insert(0, "/opt/trn_rl_repo")

import numpy as np

import concourse.bass as bass
import concourse.mybir as mybir
import concourse.tile as tile

F32 = mybir.dt.float32
BF16 = mybir.dt.bfloat16

N_CORES = 8
N, C, H, W = 2, 256, 64, 64
HW = H * W              # 4096
P = 128
CH = C // P             # 2 c-chunks
NT = HW // P            # 32 p/q tiles
Q = HW // 4             # 1024 columns per core
QT = Q // P             # 8 q tiles per block
CL = 16
SPAN = 512
EPS = 1e-12

_CACHED = {}


def _build_nc():
    from concourse import bacc

    # Bacc (not raw Bass): its compile() pass splits multi-sem waits that
    # walrus rejects ("Too many sync wait commands").
    nc = bacc.Bacc(trn_type="TRN2", num_devices=N_CORES)
    AF = mybir.ActivationFunctionType
    OP = mybir.AluOpType

    x1_d = nc.declare_dram_parameter("x1", [C, HW], F32, isOutput=False)
    x2_d = nc.declare_dram_parameter("x2", [C, HW], F32, isOutput=False)
    x_d = [x1_d, x2_d]
    ident_d = nc.declare_dram_parameter("ident", [P, P], F32, isOutput=False)
    pw1_d = nc.declare_dram_parameter("pw1t", [P, CH, CL], F32, isOutput=False)
    pw2_d = nc.declare_dram_parameter("pw2t", [P, CH, CL], F32, isOutput=False)
    pw_d = [pw1_d, pw2_d]
    pb1_d = nc.declare_dram_parameter("pb1c", [CL, 1], F32, isOutput=False)
    pb2_d = nc.declare_dram_parameter("pb2c", [CL, 1], F32, isOutput=False)
    pb_d = [pb1_d, pb2_d]
    cb1_d = nc.declare_dram_parameter("cb1c", [1, 1], F32, isOutput=False)
    cb2_d = nc.declare_dram_parameter("cb2c", [1, 1], F32, isOutput=False)
    cb_d = [cb1_d, cb2_d]
    cwb1_d = nc.declare_dram_parameter("cwb1", [P, C], F32, isOutput=False)
    cwb2_d = nc.declare_dram_parameter("cwb2", [P, C], F32, isOutput=False)
    cwb_d = [cwb1_d, cwb2_d]
    y1_d = nc.declare_dram_parameter("y1", [C, Q], F32, isOutput=True)
    y2_d = nc.declare_dram_parameter("y2", [C, Q], F32, isOutput=True)
    y_d = [y1_d, y2_d]

    groups = [[0, 1, 2, 3], [4, 5, 6, 7]]

    with tile.TileContext(nc) as tc:
        with (
            tc.tile_pool(name="const", bufs=1) as const,
            tc.tile_pool(name="persist", bufs=1) as persist,
            tc.tile_pool(name="dram", bufs=1, space="DRAM") as dram,
        ):
            # ---- constants / weights ----
            ident = const.tile([P, P], F32, name="ident", tag="ident")
            nc.sync.dma_start(ident[:], ident_d[:])
            pwt = [const.tile([P, CH, CL], F32, name=f"pwt{i}", tag=f"pwt{i}")
                   for i in range(2)]
            pbc = [const.tile([CL, 1], F32, name=f"pbc{i}", tag=f"pbc{i}")
                   for i in range(2)]
            cbb = [const.tile([P, 1], F32, name=f"cbb{i}", tag=f"cbb{i}")
                   for i in range(2)]
            cwb = [const.tile([P, C], F32, name=f"cwb{i}", tag=f"cwb{i}")
                   for i in range(2)]
            pwb = [const.tile([P, CH, CL], BF16, name=f"pwb{i}", tag=f"pwb{i}")
                   for i in range(2)]
            for i in range(2):
                nc.sync.dma_start(pwt[i][:], pw_d[i][:])
                nc.sync.dma_start(pbc[i][:], pb_d[i][:])
                nc.sync.dma_start(cbb[i][:], cb_d[i][:].to_broadcast((P, 1)))
                nc.sync.dma_start(cwb[i][:], cwb_d[i][:])
                nc.vector.tensor_copy(pwb[i][:], pwt[i][:])
            ones_c = const.tile([P, 1], F32, name="ones_c", tag="ones_c")
            ones_r = const.tile([1, P], F32, name="ones_r", tag="ones_r")
            nc.gpsimd.memset(ones_c[:], 1.0)
            nc.gpsimd.memset(ones_r[:], 1.0)

            # ---- persistent buffers ----
            xT = [persist.tile([P, NT, C], BF16, name=f"x{i}T", tag=f"x{i}T")
                  for i in range(2)]
            xR = [persist.tile([P, QT, C], F32, name=f"x{i}R", tag=f"x{i}R")
                  for i in range(2)]
            Gp = [persist.tile([CL, HW], BF16, name=f"G{i}p", tag=f"G{i}p")
                  for i in range(2)]
            fpT = [persist.tile([P, QT, C], F32, name=f"fp{i}T", tag=f"fp{i}T")
                   for i in range(2)]
            e2t = [persist.tile([P, QT], F32, name=f"e2t{i}", tag=f"e2t{i}")
                   for i in range(2)]

            # ================= Phase A =================
            with (
                tc.tile_pool(name="pa_sb", bufs=1) as pa,
                tc.tile_pool(name="pa_ps", bufs=1, space="PSUM") as pa_ps,
            ):
                for i in range(2):
                    craw = pa.tile([P, NT], F32, name=f"craw{i}", tag="craw",
                                   bufs=2)
                    for s in range(HW // SPAN):
                        xs = pa.tile([P, CH, SPAN], F32, name="xspan",
                                     tag="xspan", bufs=3)
                        nc.sync.dma_start(
                            xs[:],
                            x_d[i][:]
                            .rearrange("(a p) q -> p a q", p=P)[
                                :, :, s * SPAN:(s + 1) * SPAN
                            ],
                        )
                        # bf16 cast for the Graw matmul (gpsimd is idle)
                        xsb = pa.tile([P, CH, SPAN], BF16, name="xspanb",
                                      tag="xspanb", bufs=3)
                        nc.gpsimd.tensor_copy(xsb[:], xs[:])
                        # Graw chunk for this span (accumulate c-chunks)
                        gps = pa_ps.tile([CL, SPAN], F32, name="gps",
                                         tag="gps", bufs=2)
                        for a in range(CH):
                            nc.tensor.matmul(
                                gps[:],
                                pwb[i][:, a, :],
                                xsb[:, a, :],
                                start=(a == 0),
                                stop=(a == CH - 1),
                            )
                        nc.vector.tensor_copy(
                            Gp[i][:, s * SPAN:(s + 1) * SPAN], gps[:]
                        )
                        # transposes (exact, transpose-mode fp32)
                        for tl in range(SPAN // P):
                            t = s * (SPAN // P) + tl
                            tps = [
                                pa_ps.tile([P, P], F32, name="tps",
                                           tag="tps", bufs=4)
                                for _ in range(CH)
                            ]
                            for a in range(CH):
                                nc.tensor.transpose(
                                    tps[a][:],
                                    xs[:, a, tl * P:(tl + 1) * P],
                                    ident[:],
                                )
                                nc.vector.tensor_copy(
                                    xT[i][:, t, a * P:(a + 1) * P],
                                    tps[a][:, 0:P],
                                )
                                if t < QT:
                                    # exact fp32 residual copy from PSUM
                                    nc.vector.tensor_copy(
                                        xR[i][:, t, a * P:(a + 1) * P],
                                        tps[a][:, 0:P],
                                    )
                            # SA1 matvec: craw[:, t] = sum_c x^T[q,c]*cw[c]
                            cscr = pa.tile([P, C], F32, name="cscr",
                                           tag="cscr", bufs=2)
                            nc.vector.scalar_tensor_tensor(
                                out=cscr[:], in0=xT[i][:, t, :],
                                scalar=1.0, in1=cwb[i][:],
                                op0=OP.mult, op1=OP.mult,
                                accum_out=craw[:, t:t + 1],
                            )
                    # SA1 softmax (no max subtraction; craw ~ N(0,1))
                    e1 = pa.tile([P, NT], F32, name="e1", tag="e1", bufs=2)
                    es = pa.tile([P, 1], F32, name="es", tag="es", bufs=2)
                    nc.scalar.activation(
                        e1[:], craw[:], AF.Exp, bias=cbb[i][:], scale=1.0,
                        accum_out=es[:],
                    )
                    # partition-sum via ones-matmul, then K=1 re-broadcast
                    s_ps = pa_ps.tile([1, 1], F32, name="s_ps", tag="c1T",
                                      bufs=2)
                    nc.tensor.matmul(s_ps[:], ones_c[:], es[:])
                    rsc = pa.tile([1, 1], F32, name="rsc", tag="rsc", bufs=2)
                    nc.vector.reciprocal(rsc[:], s_ps[:])
                    rb_ps = pa_ps.tile([P, 1], F32, name="rb_ps", tag="c1T",
                                       bufs=2)
                    nc.tensor.matmul(rb_ps[:], ones_r[:], rsc[:])
                    rs = pa.tile([P, 1], F32, name="rs", tag="rs", bufs=2)
                    nc.vector.tensor_copy(rs[:], rb_ps[:])
                    c1t = pa.tile([P, NT], F32, name="c1t", tag="c1t", bufs=2)
                    nc.vector.tensor_scalar_mul(c1t[:], e1[:], rs[:])

                    # in-place scale (bf16): x_1^T = c1 * x^T
                    for t in range(NT):
                        nc.vector.tensor_scalar_mul(
                            xT[i][:, t, :], xT[i][:, t, :], c1t[:, t:t + 1]
                        )

                    # c1 row (DRAM bounce) -> 16-partition broadcast
                    c1T_ps = pa_ps.tile([32, P], F32, name="c1Tps", tag="c1T",
                                        bufs=2)
                    nc.tensor.transpose(c1T_ps[:], c1t[:], ident[:])
                    c1T = pa.tile([32, P], F32, name="c1T", tag="c1Tsb",
                                  bufs=2)
                    nc.vector.tensor_copy(c1T[:], c1T_ps[:])
                    c1row_d = dram.tile([1, HW], F32, name=f"c1row{i}",
                                        tag=f"c1row{i}")
                    nc.sync.dma_start(
                        c1row_d[:].rearrange("o (t r) -> (o t) r", t=32),
                        c1T[:],
                    )
                    c1bc = pa.tile([CL, HW], F32, name="c1bc", tag="c1bc",
                                   bufs=2)
                    nc.sync.dma_start(
                        c1bc[:], c1row_d[:].to_broadcast((CL, HW))
                    )
                    # G' = Graw * c1 + pb  (in-place on Gp, bf16 out)
                    nc.vector.tensor_tensor(
                        out=Gp[i][:], in0=Gp[i][:], in1=c1bc[:], op=OP.mult
                    )
                    nc.vector.tensor_scalar_add(Gp[i][:], Gp[i][:], pbc[i][:])

            # ================= Phase B + C =================
            with (
                tc.tile_pool(name="pb_sb", bufs=1) as pb,
                tc.tile_pool(name="pb_ps", bufs=1, space="PSUM") as pb_ps,
            ):
                ysb = [pb.tile([P, CH, Q], F32, name=f"y{i}sb", tag=f"y{i}sb")
                       for i in range(2)]
                got = [pb.tile([4, 8], F32, name=f"got{d}", tag=f"got{d}")
                       for d in range(2)]
                for d in range(2):
                    gL, gR = (Gp[0], Gp[1]) if d == 0 else (Gp[1], Gp[0])
                    acc = [
                        pb_ps.tile([P, C], F32, name=f"acc{j}", tag=f"acc{j}",
                                   bufs=1)
                        for j in range(SPAN // P)
                    ]
                    craw2 = pb.tile([P, QT], F32, name=f"craw2_{d}",
                                    tag="craw2", bufs=2)
                    for s in range(Q // SPAN):
                        for ph in range(NT // 2):  # p-tile pairs
                            aps = pb_ps.tile([P, 2, SPAN], F32, name="aps",
                                             tag="aps", bufs=2)
                            for h in range(2):
                                p = 2 * ph + h
                                nc.tensor.matmul(
                                    aps[:, h, :],
                                    gL[:, p * P:(p + 1) * P],
                                    gR[:, s * SPAN:(s + 1) * SPAN],
                                )
                            asb = pb.tile([P, 2, SPAN], BF16, name="asb",
                                          tag="asb", bufs=3)
                            nc.scalar.activation(asb[:], aps[:], AF.Tanh)
                            for h in range(2):
                                p = 2 * ph + h
                                for j in range(SPAN // P):
                                    nc.tensor.matmul(
                                        acc[j][:],
                                        asb[:, h, j * P:(j + 1) * P],
                                        xT[d][:, p, :],
                                        start=(p == 0),
                                        stop=(p == NT - 1),
                                    )
                        # post-process the 4 finished q-tiles of this span
                        ss4 = pb.tile([P, 4], F32, name="ss4", tag="ss4",
                                      bufs=2)
                        sq = [pb.tile([P, C], F32, name=f"sqt{j}", tag="sq",
                                      bufs=4) for j in range(SPAN // P)]
                        for j in range(SPAN // P):
                            nc.scalar.activation(
                                sq[j][:], acc[j][:], AF.Square,
                                accum_out=ss4[:, j:j + 1],
                            )
                        rn4 = pb.tile([P, 4], F32, name="rn4", tag="rn4",
                                      bufs=2)
                        nc.scalar.sqrt(rn4[:], ss4[:])
                        nc.vector.tensor_scalar_max(rn4[:], rn4[:], EPS)
                        nc.vector.reciprocal(rn4[:], rn4[:])
                        for j in range(SPAN // P):
                            qt = s * (SPAN // P) + j
                            tres = pb.tile([P, C], F32, name="tres",
                                           tag="tres", bufs=2)
                            nc.vector.scalar_tensor_tensor(
                                out=tres[:], in0=acc[j][:],
                                scalar=rn4[:, j:j + 1],
                                in1=xR[d][:, qt, :], op0=OP.mult, op1=OP.add,
                            )
                            nc.scalar.activation(
                                fpT[d][:, qt, :], tres[:], AF.Relu
                            )
                            sq2 = pb.tile([P, C], F32, name="sq2", tag="sq",
                                          bufs=4)
                            nc.vector.scalar_tensor_tensor(
                                out=sq2[:], in0=fpT[d][:, qt, :],
                                scalar=1.0, in1=cwb[d][:],
                                op0=OP.mult, op1=OP.mult,
                                accum_out=craw2[:, qt:qt + 1],
                            )
                    # SA2 local stats + AllGather
                    es2 = pb.tile([P, 1], F32, name="es2", tag="es2", bufs=2)
                    nc.scalar.activation(
                        e2t[d][:], craw2[:], AF.Exp, bias=cbb[d][:],
                        scale=1.0, accum_out=es2[:],
                    )
                    s2_ps = pb_ps.tile([1, 1], F32, name="s2_ps", tag="aps",
                                       bufs=2)
                    nc.tensor.matmul(s2_ps[:], ones_c[:], es2[:])
                    pad = pb.tile([1, 8], F32, name="pad", tag="pad", bufs=2)
                    nc.gpsimd.memset(pad[:], 0.0)
                    nc.vector.tensor_copy(pad[:, 0:1], s2_ps[:])
                    cc_in = dram.tile([1, 8], F32, name=f"ccin{d}",
                                      tag=f"ccin{d}")
                    cc_out = dram.tile([4, 8], F32, name=f"ccout{d}",
                                       tag=f"ccout{d}")
                    nc.sync.dma_start(cc_in[:], pad[:])
                    nc.gpsimd.collective_compute(
                        "AllGather", mybir.AluOpType.bypass,
                        replica_groups=groups, ins=[cc_in[:]],
                        outs=[cc_out[:]],
                    )
                    nc.sync.dma_start(got[d][:], cc_out[:])

                # ---- finalize ----
                for d in range(2):
                    stot_ps = pb_ps.tile([1, 8], F32, name=f"stot{d}",
                                         tag="aps", bufs=2)
                    nc.tensor.matmul(stot_ps[:], ones_c[0:4, :], got[d][:])
                    rq = pb.tile([1, 1], F32, name=f"rq{d}", tag="rq", bufs=2)
                    nc.vector.reciprocal(rq[:], stot_ps[:, 0:1])
                    rb_d = dram.tile([1, 1], F32, name=f"rb{d}", tag=f"rb{d}")
                    nc.sync.dma_start(rb_d[:], rq[:])
                    rb = pb.tile([P, 1], F32, name=f"rb{d}", tag="rb", bufs=2)
                    nc.sync.dma_start(rb[:], rb_d[:].to_broadcast((P, 1)))
                    scl = pb.tile([P, QT], F32, name=f"scl{d}", tag="scl",
                                  bufs=2)
                    nc.vector.tensor_scalar_mul(scl[:], e2t[d][:], rb[:])
                    y_dd = y_d[d]
                    for qt in range(QT):
                        fin = pb.tile([P, C], F32, name="fin", tag="fin",
                                      bufs=3)
                        nc.vector.tensor_scalar_mul(
                            fin[:], fpT[d][:, qt, :], scl[:, qt:qt + 1]
                        )
                        for a in range(CH):
                            ytp = pb_ps.tile([P, P], F32, name="ytp",
                                             tag="aps", bufs=2)
                            nc.tensor.transpose(
                                ytp[:], fin[:, a * P:(a + 1) * P], ident[:]
                            )
                            nc.vector.tensor_copy(
                                ysb[d][:, a, qt * P:(qt + 1) * P], ytp[:]
                            )
                    for a in range(CH):
                        nc.sync.dma_start(
                            y_dd[a * P:(a + 1) * P, :], ysb[d][:, a, :]
                        )
    if not nc.is_finalized():
        nc.finalize()
    return nc


def _get_nc():
    if "nc" not in _CACHED:
        _CACHED["nc"] = _build_nc()
    return _CACHED["nc"]


def _prep_inputs(f1, f2, pw1, pb1, pw2, pb2, cw1, cb1, cw2, cb2):
    f1 = np.asarray(f1, np.float32).reshape(N, C, HW)
    f2 = np.asarray(f2, np.float32).reshape(N, C, HW)

    def pwt(pw):
        w = np.asarray(pw, np.float32).reshape(CL, CH, P)  # [k, a, p]
        return np.ascontiguousarray(w.transpose(2, 1, 0))  # [p, a, k]

    common = {
        "ident": np.eye(P, dtype=np.float32),
        "pw1t": pwt(pw1), "pw2t": pwt(pw2),
        "pb1c": np.asarray(pb1, np.float32).reshape(CL, 1),
        "pb2c": np.asarray(pb2, np.float32).reshape(CL, 1),
        "cb1c": np.asarray(cb1, np.float32).reshape(1, 1),
        "cb2c": np.asarray(cb2, np.float32).reshape(1, 1),
        "cwb1": np.broadcast_to(
            np.asarray(cw1, np.float32).reshape(1, C), (P, C)
        ).copy(),
        "cwb2": np.broadcast_to(
            np.asarray(cw2, np.float32).reshape(1, C), (P, C)
        ).copy(),
    }
    in_maps = []
    for core in range(N_CORES):
        n, b = divmod(core, 4)
        m = dict(common)
        m["x1"] = np.ascontiguousarray(np.roll(f1[n], -b * Q, axis=1))
        m["x2"] = np.ascontiguousarray(np.roll(f2[n], -b * Q, axis=1))
        in_maps.append(m)
    return in_maps


def _assemble(results):
    o1 = np.empty((N, C, HW), np.float32)
    o2 = np.empty((N, C, HW), np.float32)
    for core in range(N_CORES):
        n, b = divmod(core, 4)
        o1[n][:, b * Q:(b + 1) * Q] = results[core]["y1"]
        o2[n][:, b * Q:(b + 1) * Q] = results[core]["y2"]
    return o1.reshape(N, C, H, W), o2.reshape(N, C, H, W)


def run(trace=False, **inputs):
    from concourse.bass_utils import run_bass_kernel_spmd

    nc = _get_nc()
    in_maps = _prep_inputs(**inputs)
    res = run_bass_kernel_spmd(
        nc, in_maps, core_ids=list(range(N_CORES)), trace=trace
    )
    return _assemble(res.results), res


def kernel(**inputs):
    out, _ = run(trace=False, **inputs)
    return out


# revision 34
# speedup vs baseline: 1.4082x; 1.1295x over previous
"""Trainium2 Bass kernel for nn_BidirectionalRead (N=2, C=256, H=W=64).

Reference (per sample; f1,f2 as [C=256, HW=4096]):
  SA1:  c_i = softmax(cw_i @ f_i + cb_i) over HW;  f_i1 = c_i * f_i
  CA:   G_i = pw_i @ f_i1 + pb_i  [16, HW]
        B[p,q] = tanh(sum_k G1[k,p] G2[k,q]);  A = B^T
        f1_hat = l2norm_c(f1_1 @ B); f2_hat = l2norm_c(f2_1 @ A)
  fp_i  = relu(f_i_hat + f_i)
  SA2:  out_i = softmax(cw_i @ fp_i + cb_i) * fp_i

Sharding: 8 cores = 2 samples x 4 q-blocks of 1024 columns. Inputs are
np.roll'ed per core so identical SPMD code always computes output columns
0..1023 of its rotated view. SA2 softmax denominators are exchanged with
one tiny AllGather per direction over groups [[0-3],[4-7]].

v3 notes:
 - cw rides as a 17th row of pw^T, so the SA1 matvec is a free by-product
   of the (plain fp32) Graw matmul; SA1 softmax runs on the [1, 4096] row
   and is redistributed via DRAM-bounce DMAs.
 - x transposed on the PE (transpose-mode fp32, exact); PSUM->SBUF copies
   write bf16 (split DVE/ACT); residual block copied exact in fp32.
 - Main/A-gen matmuls all bf16 (1 cyc/row + FWL); f_hat emerges
   TRANSPOSED [q, c] so l2norm/residual/SA2-matvec are free-dim ops.
 - tanh over merged [128, 2x512] pairs; finalize pre-scales by the local
   exp factor so only a scalar 1/S multiply trails the AllGather.
"""

import sys

sys.path.insert(0, "/opt/trn_rl_repo")

import numpy as np

import concourse.bass as bass
import concourse.mybir as mybir
import concourse.tile as tile

F32 = mybir.dt.float32
BF16 = mybir.dt.bfloat16

N_CORES = 8
N, C, H, W = 2, 256, 64, 64
HW = H * W              # 4096
P = 128
CH = C // P             # 2 c-chunks
NT = HW // P            # 32 p/q tiles
Q = HW // 4             # 1024 columns per core
QT = Q // P             # 8 q tiles per block
CL = 16
RIDER = 32              # partition of the cw rider row (32-aligned)
CL1 = RIDER + 1         # pw rows + zero pad + the cw rider row
SPAN = 512
EPS = 1e-12

_CACHED = {}


def _build_nc():
    from concourse import bacc

    # Bacc (not raw Bass): its compile() pass splits multi-sem waits that
    # walrus rejects ("Too many sync wait commands").
    nc = bacc.Bacc(trn_type="TRN2", num_devices=N_CORES)
    AF = mybir.ActivationFunctionType
    OP = mybir.AluOpType

    x1_d = nc.declare_dram_parameter("x1", [C, HW], F32, isOutput=False)
    x2_d = nc.declare_dram_parameter("x2", [C, HW], F32, isOutput=False)
    x_d = [x1_d, x2_d]
    ident_d = nc.declare_dram_parameter("ident", [P, P], F32, isOutput=False)
    # pw^T with cw as row CL: [p, a, 17]
    pw1_d = nc.declare_dram_parameter("pw1t", [P, CH, CL1], F32, isOutput=False)
    pw2_d = nc.declare_dram_parameter("pw2t", [P, CH, CL1], F32, isOutput=False)
    pw_d = [pw1_d, pw2_d]
    pb1_d = nc.declare_dram_parameter("pb1c", [CL, 1], F32, isOutput=False)
    pb2_d = nc.declare_dram_parameter("pb2c", [CL, 1], F32, isOutput=False)
    pb_d = [pb1_d, pb2_d]
    cb1_d = nc.declare_dram_parameter("cb1c", [1, 1], F32, isOutput=False)
    cb2_d = nc.declare_dram_parameter("cb2c", [1, 1], F32, isOutput=False)
    cb_d = [cb1_d, cb2_d]
    cwb1_d = nc.declare_dram_parameter("cwb1", [P, C], F32, isOutput=False)
    cwb2_d = nc.declare_dram_parameter("cwb2", [P, C], F32, isOutput=False)
    cwb_d = [cwb1_d, cwb2_d]
    y1_d = nc.declare_dram_parameter("y1", [C, Q], F32, isOutput=True)
    y2_d = nc.declare_dram_parameter("y2", [C, Q], F32, isOutput=True)
    y_d = [y1_d, y2_d]

    groups = [[0, 1, 2, 3], [4, 5, 6, 7]]

    with tile.TileContext(nc) as tc:
        with (
            tc.tile_pool(name="const", bufs=1) as const,
            tc.tile_pool(name="persist", bufs=1) as persist,
            tc.tile_pool(name="dram", bufs=1, space="DRAM") as dram,
        ):
            # ---- constants / weights ----
            ident = const.tile([P, P], F32, name="ident", tag="ident")
            nc.sync.dma_start(ident[:], ident_d[:])
            pwt = [const.tile([P, CH, CL1], F32, name=f"pwt{i}", tag=f"pwt{i}")
                   for i in range(2)]
            pbc = [const.tile([CL, 1], F32, name=f"pbc{i}", tag=f"pbc{i}")
                   for i in range(2)]
            cbb = [const.tile([P, 1], F32, name=f"cbb{i}", tag=f"cbb{i}")
                   for i in range(2)]
            cbs = [const.tile([1, 1], F32, name=f"cbs{i}", tag=f"cbs{i}")
                   for i in range(2)]
            cwbb = [const.tile([P, C], BF16, name=f"cwbb{i}", tag=f"cwbb{i}")
                    for i in range(2)]
            cwbf = [const.tile([P, C], F32, name=f"cwbf{i}", tag=f"cwbf{i}")
                    for i in range(2)]
            for i in range(2):
                nc.sync.dma_start(pwt[i][:], pw_d[i][:])
                nc.sync.dma_start(pbc[i][:], pb_d[i][:])
                nc.sync.dma_start(cbb[i][:], cb_d[i][:].to_broadcast((P, 1)))
                nc.sync.dma_start(cbs[i][:], cb_d[i][:])
                nc.sync.dma_start(cwbf[i][:], cwb_d[i][:])
                nc.vector.tensor_copy(cwbb[i][:], cwbf[i][:])

            # ---- persistent buffers ----
            xT = [persist.tile([P, NT, C], BF16, name=f"x{i}T", tag=f"x{i}T")
                  for i in range(2)]
            xR = [persist.tile([P, QT, C], F32, name=f"x{i}R", tag=f"x{i}R")
                  for i in range(2)]
            # rows 0..15: Graw; row 16: SA1 matvec row (craw) — fp32 raw
            Gp = [persist.tile([CL1, HW], F32, name=f"G{i}p", tag=f"G{i}p")
                  for i in range(2)]
            # scaled G' in bf16 (matmul operand)
            Gb = [persist.tile([CL, HW], BF16, name=f"G{i}b", tag=f"G{i}b")
                  for i in range(2)]
            fpT = [persist.tile([P, QT, C], F32, name=f"fp{i}T", tag=f"fp{i}T")
                   for i in range(2)]
            e2t = [persist.tile([P, QT], F32, name=f"e2t{i}", tag=f"e2t{i}")
                   for i in range(2)]
            ysb = [persist.tile([P, CH, Q], F32, name=f"y{i}sb",
                                tag=f"y{i}sb") for i in range(2)]

            # ================= Phase A =================
            with (
                tc.tile_pool(name="pa_sb", bufs=1) as pa,
                tc.tile_pool(name="pa_ps", bufs=1, space="PSUM") as pa_ps,
            ):
                for i in range(2):
                    for s in range(HW // SPAN):
                        xs = pa.tile([P, CH, SPAN], F32, name="xspan",
                                     tag="xspan", bufs=3)
                        nc.sync.dma_start(
                            xs[:],
                            x_d[i][:]
                            .rearrange("(a p) q -> p a q", p=P)[
                                :, :, s * SPAN:(s + 1) * SPAN
                            ],
                        )
                        # Graw chunk + SA1 matvec row (fp32, plain)
                        gps = pa_ps.tile([CL1, SPAN], F32, name="gps",
                                         tag="gps", bufs=2)
                        for a in range(CH):
                            nc.tensor.matmul(
                                gps[:],
                                pwt[i][:, a, :],
                                xs[:, a, :],
                                start=(a == 0),
                                stop=(a == CH - 1),
                            )
                        nc.vector.tensor_copy(
                            Gp[i][:, s * SPAN:(s + 1) * SPAN], gps[:]
                        )
                        # transposes (exact, transpose-mode fp32);
                        # PSUM->SBUF bf16 copies split between DVE and ACT
                        for tl in range(SPAN // P):
                            t = s * (SPAN // P) + tl
                            tps = [
                                pa_ps.tile([P, P], F32, name="tps",
                                           tag="tps", bufs=4)
                                for _ in range(CH)
                            ]
                            for a in range(CH):
                                nc.tensor.transpose(
                                    tps[a][:],
                                    xs[:, a, tl * P:(tl + 1) * P],
                                    ident[:],
                                )
                                dst = xT[i][:, t, a * P:(a + 1) * P]
                                if a == 0:
                                    nc.vector.tensor_copy(dst, tps[a][:])
                                else:
                                    nc.scalar.copy(dst, tps[a][:])
                                if t < QT:
                                    # exact fp32 residual copy from PSUM
                                    nc.vector.tensor_copy(
                                        xR[i][:, t, a * P:(a + 1) * P],
                                        tps[a][:],
                                    )
                    # ---- SA1 softmax on the [1, HW] rider row ----
                    e_row = pa.tile([1, HW], F32, name="e_row", tag="e_row",
                                    bufs=2)
                    s_acc = pa.tile([1, 1], F32, name="s_acc", tag="s_acc",
                                    bufs=2)
                    nc.scalar.activation(
                        e_row[:], Gp[i][RIDER:CL1, :], AF.Exp,
                        bias=cbs[i][:], scale=1.0, accum_out=s_acc[:],
                    )
                    rsc = pa.tile([1, 1], F32, name="rsc", tag="rsc", bufs=2)
                    nc.vector.reciprocal(rsc[:], s_acc[:])
                    e_row_d = dram.tile([1, HW], F32, name=f"erow{i}",
                                        tag=f"erow{i}")
                    rs_d = dram.tile([1, 1], F32, name=f"rs{i}", tag=f"rs{i}")
                    nc.sync.dma_start(e_row_d[:], e_row[:])
                    nc.sync.dma_start(rs_d[:], rsc[:])
                    # tiled form [128, NT] (scattered 4B DMA, small) and
                    # 16-row broadcast for the G' scale
                    e_til = pa.tile([P, NT], F32, name="e_til", tag="e_til",
                                    bufs=2)
                    nc.sync.dma_start(
                        e_til[:],
                        e_row_d[:].rearrange("o (t r) -> (o r) t", t=NT),
                    )
                    rs128 = pa.tile([P, 1], F32, name="rs128", tag="rs128",
                                    bufs=2)
                    nc.sync.dma_start(rs128[:], rs_d[:].to_broadcast((P, 1)))
                    c1til = pa.tile([P, NT], F32, name="c1til", tag="c1til",
                                    bufs=2)
                    nc.vector.tensor_scalar_mul(c1til[:], e_til[:], rs128[:])
                    ebc = pa.tile([CL, HW], F32, name="ebc", tag="ebc",
                                  bufs=1)
                    nc.sync.dma_start(ebc[:], e_row_d[:].to_broadcast((CL, HW)))
                    rs16 = pa.tile([CL, 1], F32, name="rs16", tag="rs16",
                                   bufs=2)
                    nc.sync.dma_start(rs16[:], rs_d[:].to_broadcast((CL, 1)))

                    # in-place scale (bf16): x_1^T = c1 * x^T
                    for t in range(NT):
                        nc.vector.tensor_scalar_mul(
                            xT[i][:, t, :], xT[i][:, t, :], c1til[:, t:t + 1]
                        )
                    # G' = Graw * (e/S) + pb  (bf16 matmul operand)
                    nc.vector.scalar_tensor_tensor(
                        out=Gb[i][:], in0=Gp[i][0:CL, :],
                        scalar=rs16[:], in1=ebc[:],
                        op0=OP.mult, op1=OP.mult,
                    )
                    nc.vector.tensor_scalar_add(
                        Gb[i][:], Gb[i][:], pbc[i][:]
                    )

            # ================= Phase B + C =================
            with (
                tc.tile_pool(name="pb_sb", bufs=1) as pb,
                tc.tile_pool(name="pb_ps", bufs=1, space="PSUM") as pb_ps,
            ):
                got = [pb.tile([4, 8], F32, name=f"got{d}", tag=f"got{d}")
                       for d in range(2)]
                ones_c = pb.tile([P, 1], F32, name="ones_c", tag="ones_c")
                nc.gpsimd.memset(ones_c[:], 1.0)
                for d in range(2):
                    gL, gR = (Gb[0], Gb[1]) if d == 0 else (Gb[1], Gb[0])
                    acc = [
                        pb_ps.tile([P, C], F32, name=f"acc{j}", tag=f"acc{j}",
                                   bufs=1)
                        for j in range(SPAN // P)
                    ]
                    craw2 = pb.tile([P, QT], F32, name=f"craw2_{d}",
                                    tag="craw2", bufs=2)
                    for s in range(Q // SPAN):
                        for ph in range(NT // 2):  # p-tile pairs
                            aps = pb_ps.tile([P, 2, SPAN], F32, name="aps",
                                             tag="aps", bufs=2)
                            for h in range(2):
                                p = 2 * ph + h
                                nc.tensor.matmul(
                                    aps[:, h, :],
                                    gL[:, p * P:(p + 1) * P],
                                    gR[:, s * SPAN:(s + 1) * SPAN],
                                )
                            asb = pb.tile([P, 2, SPAN], BF16, name="asb",
                                          tag="asb", bufs=3)
                            nc.scalar.activation(asb[:], aps[:], AF.Tanh)
                            for h in range(2):
                                p = 2 * ph + h
                                for j in range(SPAN // P):
                                    nc.tensor.matmul(
                                        acc[j][:],
                                        asb[:, h, j * P:(j + 1) * P],
                                        xT[d][:, p, :],
                                        start=(p == 0),
                                        stop=(p == NT - 1),
                                    )
                        # post-process the 4 finished q-tiles of this span
                        ss4 = pb.tile([P, 4], F32, name="ss4", tag="ss4",
                                      bufs=2)
                        sq = [pb.tile([P, C], F32, name=f"sqt{j}", tag="sq",
                                      bufs=4) for j in range(SPAN // P)]
                        for j in range(SPAN // P):
                            nc.scalar.activation(
                                sq[j][:], acc[j][:], AF.Square,
                                accum_out=ss4[:, j:j + 1],
                            )
                        rn4 = pb.tile([P, 4], F32, name="rn4", tag="rn4",
                                      bufs=2)
                        nc.scalar.sqrt(rn4[:], ss4[:])
                        nc.vector.tensor_scalar_max(rn4[:], rn4[:], EPS)
                        nc.vector.reciprocal(rn4[:], rn4[:])
                        for j in range(SPAN // P):
                            qt = s * (SPAN // P) + j
                            tres = pb.tile([P, C], F32, name="tres",
                                           tag="tres", bufs=2)
                            nc.vector.scalar_tensor_tensor(
                                out=tres[:], in0=acc[j][:],
                                scalar=rn4[:, j:j + 1],
                                in1=xR[d][:, qt, :], op0=OP.mult, op1=OP.add,
                            )
                            nc.scalar.activation(
                                fpT[d][:, qt, :], tres[:], AF.Relu
                            )
                            sq2 = pb.tile([P, C], F32, name="sq2", tag="sq",
                                          bufs=4)
                            nc.vector.scalar_tensor_tensor(
                                out=sq2[:], in0=fpT[d][:, qt, :],
                                scalar=1.0, in1=cwbf[d][:],
                                op0=OP.mult, op1=OP.mult,
                                accum_out=craw2[:, qt:qt + 1],
                            )
                    # SA2 local stats + AllGather
                    es2 = pb.tile([P, 1], F32, name="es2", tag="es2", bufs=2)
                    nc.scalar.activation(
                        e2t[d][:], craw2[:], AF.Exp, bias=cbb[d][:],
                        scale=1.0, accum_out=es2[:],
                    )
                    s2_ps = pb_ps.tile([1, 1], F32, name="s2_ps", tag="aps",
                                       bufs=2)
                    nc.tensor.matmul(s2_ps[:], ones_c[:], es2[:])
                    pad = pb.tile([1, 8], F32, name="pad", tag="pad", bufs=2)
                    nc.gpsimd.memset(pad[:], 0.0)
                    nc.vector.tensor_copy(pad[:, 0:1], s2_ps[:])
                    cc_in = dram.tile([1, 8], F32, name=f"ccin{d}",
                                      tag=f"ccin{d}")
                    cc_out = dram.tile([4, 8], F32, name=f"ccout{d}",
                                       tag=f"ccout{d}")
                    nc.sync.dma_start(cc_in[:], pad[:])
                    nc.gpsimd.collective_compute(
                        "AllGather", mybir.AluOpType.bypass,
                        replica_groups=groups, ins=[cc_in[:]],
                        outs=[cc_out[:]],
                    )
                    nc.sync.dma_start(got[d][:], cc_out[:])

                    # pre-collective finalize: scale by local exp factor,
                    # transpose back to [c, q], stage into ysb
                    for qt in range(QT):
                        fin = pb.tile([P, C], F32, name="fin", tag="fin",
                                      bufs=3)
                        nc.vector.tensor_scalar_mul(
                            fin[:], fpT[d][:, qt, :], e2t[d][:, qt:qt + 1]
                        )
                        for a in range(CH):
                            ytp = pb_ps.tile([P, P], F32, name="ytp",
                                             tag="aps", bufs=2)
                            nc.tensor.transpose(
                                ytp[:], fin[:, a * P:(a + 1) * P], ident[:]
                            )
                            nc.vector.tensor_copy(
                                ysb[d][:, a, qt * P:(qt + 1) * P], ytp[:]
                            )

                # ---- post-collective: scalar 1/S multiply + store ----
                for d in range(2):
                    stot_ps = pb_ps.tile([1, 8], F32, name=f"stot{d}",
                                         tag="aps", bufs=2)
                    nc.tensor.matmul(stot_ps[:], ones_c[0:4, :], got[d][:])
                    rq = pb.tile([1, 1], F32, name=f"rq{d}", tag="rq", bufs=2)
                    nc.vector.reciprocal(rq[:], stot_ps[:, 0:1])
                    rb_d = dram.tile([1, 1], F32, name=f"rbd{d}",
                                     tag=f"rbd{d}")
                    nc.sync.dma_start(rb_d[:], rq[:])
                    rb = pb.tile([P, 1], F32, name=f"rb{d}", tag="rb", bufs=2)
                    nc.sync.dma_start(rb[:], rb_d[:].to_broadcast((P, 1)))
                    for a in range(CH):
                        nc.vector.tensor_scalar_mul(
                            ysb[d][:, a, :], ysb[d][:, a, :], rb[:]
                        )
                        nc.sync.dma_start(
                            y_d[d][a * P:(a + 1) * P, :], ysb[d][:, a, :]
                        )
    if not nc.is_finalized():
        nc.finalize()
    return nc


def _get_nc():
    if "nc" not in _CACHED:
        _CACHED["nc"] = _build_nc()
    return _CACHED["nc"]


def _prep_inputs(f1, f2, pw1, pb1, pw2, pb2, cw1, cb1, cw2, cb2):
    f1 = np.asarray(f1, np.float32).reshape(N, C, HW)
    f2 = np.asarray(f2, np.float32).reshape(N, C, HW)

    def pwt(pw, cw):
        w = np.zeros((CL1, C), np.float32)
        w[0:CL] = np.asarray(pw, np.float32).reshape(CL, C)
        w[RIDER] = np.asarray(cw, np.float32).reshape(C)
        w = w.reshape(CL1, CH, P)          # [k, a, p]
        return np.ascontiguousarray(w.transpose(2, 1, 0))  # [p, a, k]

    common = {
        "ident": np.eye(P, dtype=np.float32),
        "pw1t": pwt(pw1, cw1), "pw2t": pwt(pw2, cw2),
        "pb1c": np.asarray(pb1, np.float32).reshape(CL, 1),
        "pb2c": np.asarray(pb2, np.float32).reshape(CL, 1),
        "cb1c": np.asarray(cb1, np.float32).reshape(1, 1),
        "cb2c": np.asarray(cb2, np.float32).reshape(1, 1),
        "cwb1": np.broadcast_to(
            np.asarray(cw1, np.float32).reshape(1, C), (P, C)
        ).copy(),
        "cwb2": np.broadcast_to(
            np.asarray(cw2, np.float32).reshape(1, C), (P, C)
        ).copy(),
    }
    in_maps = []
    for core in range(N_CORES):
        n, b = divmod(core, 4)
        m = dict(common)
        m["x1"] = np.ascontiguousarray(np.roll(f1[n], -b * Q, axis=1))
        m["x2"] = np.ascontiguousarray(np.roll(f2[n], -b * Q, axis=1))
        in_maps.append(m)
    return in_maps


def _assemble(results):
    o1 = np.empty((N, C, HW), np.float32)
    o2 = np.empty((N, C, HW), np.float32)
    for core in range(N_CORES):
        n, b = divmod(core, 4)
        o1[n][:, b * Q:(b + 1) * Q] = results[core]["y1"]
        o2[n][:, b * Q:(b + 1) * Q] = results[core]["y2"]
    return o1.reshape(N, C, H, W), o2.reshape(N, C, H, W)


def run(trace=False, **inputs):
    from concourse.bass_utils import run_bass_kernel_spmd

    nc = _get_nc()
    in_maps = _prep_inputs(**inputs)
    res = run_bass_kernel_spmd(
        nc, in_maps, core_ids=list(range(N_CORES)), trace=trace
    )
    return _assemble(res.results), res


def kernel(**inputs):
    out, _ = run(trace=False, **inputs)
    return out
